# revision 1
# baseline (speedup 1.0000x reference)
"""Masked L1 loss (per-(b,c) normalized) on 8 Trainium2 NeuronCores.

Layout: batch-dim data parallel. Core i takes batches [2i, 2i+2) of the
[16, 64, 128, 128] inputs -> a [128, 16384] shard (partition = (b, c) pair,
free = h*w). The kernel is DMA-bound, so everything is built around the DMA
stream:

  - pre and -gt are repacked on the host into one tile-major tensor
    (per tile: [pre_i | ng_i] blocks, ng = -gt so combines are adds). All
    input DMAs use the gpsimd SWDGE path (the only one that can downcast
    in flight); descriptor generation is serial (~1 us per DMA) so the DMA
    count is kept small and per-DMA transfers large.
  - every transfer downcasts f32 -> bf16 inline: HBM reads are unchanged
    but the SBUF-side stream halves and all on-chip operands are 16-bit
    (2x DVE tensor_tensor mode). Loss tolerance is 2e-2; bf16 + fp32
    accumulation gives ~5e-6.
  - the SUBTRACT itself runs inside the DMA engine where possible: for
    mid-size tiles a CCE accumulate pair computes sd = bf16(pre) + bf16(ng)
    (dst += src during the second transfer). The CCE datapath handles at
    most 2048 elements per descriptor, so the first 4096-wide tile is
    split into two half pairs and the big/small remaining tiles fall back
    to one packed DMA + a DVE add.
  - per tile DVE then runs at most two 2x-mode tensor_tensor ops; the l1
    partial sum(|y|) = sum(|d|*mask) comes from ACT activation(Abs,
    accum_out) consuming y = sd*mask. ACT is a pure sink, so the
    cross-engine hop never stalls the pipeline. The last two small tiles
    use the all-DVE tensor_reduce(add, apply_absolute_value) instead, so
    the post-last-byte chain stays on one engine.
  - the bf16 mask stays resident in SBUF (32 KiB/partition), fully fetched
    by pg tile 1 in four 4096-column chunks; counts (sum of 0/1 mask,
    exact in bf16) run early over those chunks, alternating DVE
    tensor_reduce / ACT activation(Copy, accum_out), so no count gates the
    tail.
  - tile sizes shrink geometrically (4096 -> 256): the final DMA gates only
    a tiny DVE chain and the single output DMA (all partials in one fp32
    tile).
Host: loss = sum(l1 / max(ct, 1)) / B.
"""

import sys

if "/opt/trn_rl_repo" not in sys.path:
    sys.path.insert(0, "/opt/trn_rl_repo")

import numpy as np

B, C, H, W = 16, 64, 128, 128
N_CORES = 8
BPC = B // N_CORES          # batches per core = 2
P = BPC * C                 # partitions per core = 128 (one (b,c) pair each)
HW = H * W                  # 16384 free elements per partition

SIZES = [4096, 4096, 2048, 2048, 1664, 1024, 768, 512, 128]   # sum = 16384
NT = len(SIZES)
OFFS = [sum(SIZES[:i]) for i in range(NT)]
N_DVE_TAIL = 2              # last tiles reduce on DVE (no ACT in the tail)
ACCUM_TILES = {2, 3, 4}     # subtract fused into the DMA (CCE adder) for
                            # these tiles; must be <= 2048 cols (CCE element
                            # limit per descriptor)
SPLIT_TILES = {0}           # oversize tiles whose DMA is split into two
                            # CCE-safe half pairs (compute tiling unchanged)
SPLIT_RED = {4: 0.2, 6: 0.5}   # these tiles' |y| reductions are split: the
                               # given fraction on a DVE tensor_reduce, the
                               # rest on ACT absacc (extra out columns,
                               # summed into l1 on the host)
NXT = len(SPLIT_RED)

# mask chunk DMAs (lo, hi, after_pg_tile): the whole mask lands by pg1 so
# counts and the mult deps never gate the tail
MASK_CHUNKS = [
    (0, 4096, 0),
    (4096, 8192, 0),
    (8192, 12288, 1),
    (12288, 16384, 1),
]

# count chunks (lo, hi, engine, emit_after_tile); ranges must be fully
# DMA'd by their emit point
COUNT_CHUNKS = [
    (0, 4096, "act", 0),
    (4096, 8192, "dve", 1),
    (8192, 12288, "act", 2),
    (12288, 16384, "dve", 3),
]
NCC = len(COUNT_CHUNKS)

_CACHE = {}


def _build():
    key = "nc"
    if key in _CACHE:
        return _CACHE[key]

    import concourse.bacc as bacc
    import concourse.mybir as mybir
    from concourse.tile import TileContext

    f32 = mybir.dt.float32
    bf16 = mybir.dt.bfloat16
    Alu = mybir.AluOpType
    Act = mybir.ActivationFunctionType

    nc = bacc.Bacc(
        "TRN2",
        target_bir_lowering=False,
        debug=False,
        enable_asserts=False,
        num_devices=N_CORES,
    )

    pgin = nc.dram_tensor("pgin", [P, 2 * HW], f32, kind="ExternalInput").ap()
    mask = nc.dram_tensor("mask", [P, HW], f32, kind="ExternalInput").ap()
    out = nc.dram_tensor("out", [P, NT + NCC + NXT], f32, kind="ExternalOutput").ap()

    with TileContext(nc) as tc:
        with (
            tc.tile_pool(name="pg", bufs=4) as pg,
            tc.tile_pool(name="mp", bufs=1) as mp,
            tc.tile_pool(name="work", bufs=4) as work,
            tc.tile_pool(name="acc", bufs=1) as accp,
        ):
            acc = accp.tile([P, NT + NCC + NXT], f32, tag="acc")
            l1p = acc[:, 0:NT]
            ctp = acc[:, NT : NT + NCC]
            l1x = acc[:, NT + NCC : NT + NCC + NXT]
            xi = 0
            trash = accp.tile([P, 4096], bf16, tag="trash")
            tmr = mp.tile([P, HW], bf16, tag="mask")   # resident bf16 mask

            def emit_counts(after_tile):
                for ci, (lo, hi, eng, ready) in enumerate(COUNT_CHUNKS):
                    if ready != after_tile:
                        continue
                    if eng == "dve":
                        nc.vector.tensor_reduce(
                            out=ctp[:, ci : ci + 1],
                            in_=tmr[:, lo:hi],
                            axis=mybir.AxisListType.X,
                            op=Alu.add,
                        )
                    else:
                        nc.scalar.activation(
                            out=trash[:, : hi - lo],
                            in_=tmr[:, lo:hi],
                            func=Act.Copy,
                            accum_out=ctp[:, ci : ci + 1],
                        )

            for i in range(NT):
                s, o = SIZES[i], OFFS[i]
                sd = work.tile([P, s], bf16, tag="sd", name=f"sd{i}")

                # DMA order: pg_i (one packed DMA, or a CCE pair computing
                # sd = bf16(pre) + bf16(ng) in the DMA engine for mid-size
                # tiles), then any mask chunks scheduled after it. The final
                # DMA is the last (tiny) pg tile, gating one short DVE chain.
                if i in SPLIT_TILES:
                    h = s // 2
                    for k in (0, 1):
                        nc.gpsimd.dma_start(
                            out=sd[:, k * h : (k + 1) * h],
                            in_=pgin[:, 2 * o + k * h : 2 * o + (k + 1) * h],
                        )
                        nc.gpsimd.dma_start(
                            out=sd[:, k * h : (k + 1) * h],
                            in_=pgin[:, 2 * o + s + k * h : 2 * o + s + (k + 1) * h],
                            accum_op=Alu.add,
                        )
                elif i in ACCUM_TILES:
                    nc.gpsimd.dma_start(out=sd, in_=pgin[:, 2 * o : 2 * o + s])
                    nc.gpsimd.dma_start(
                        out=sd,
                        in_=pgin[:, 2 * o + s : 2 * o + 2 * s],
                        accum_op=Alu.add,
                    )
                else:
                    xt = pg.tile([P, 2 * s], bf16, tag="pg", name=f"xt{i}")
                    nc.gpsimd.dma_start(out=xt, in_=pgin[:, 2 * o : 2 * o + 2 * s])
                for lo, hi, after in MASK_CHUNKS:
                    if after == i:
                        nc.gpsimd.dma_start(out=tmr[:, lo:hi], in_=mask[:, lo:hi])

                emit_counts(i)

                # DVE 2x-mode TTs; ACT (or DVE for tail tiles) reduces.
                # ng = -gt on the host, so the combine op is an add.
                y = work.tile([P, s], bf16, tag="y", name=f"y{i}")
                if i not in ACCUM_TILES and i not in SPLIT_TILES:
                    nc.vector.tensor_tensor(
                        out=sd, in0=xt[:, 0:s], in1=xt[:, s : 2 * s], op=Alu.add
                    )
                nc.vector.tensor_tensor(
                    out=y, in0=sd, in1=tmr[:, o : o + s], op=Alu.mult
                )
                if i in SPLIT_RED:
                    h = int(s * SPLIT_RED[i]) // 64 * 64
                    nc.vector.tensor_reduce(
                        out=l1p[:, i : i + 1],
                        in_=y[:, 0:h],
                        axis=mybir.AxisListType.X,
                        op=Alu.add,
                        apply_absolute_value=True,
                    )
                    nc.scalar.activation(
                        out=trash[:, : s - h],
                        in_=y[:, h:s],
                        func=Act.Abs,
                        accum_out=l1x[:, xi : xi + 1],
                    )
                    xi += 1
                elif i < NT - N_DVE_TAIL:
                    nc.scalar.activation(
                        out=trash[:, :s],
                        in_=y,
                        func=Act.Abs,
                        accum_out=l1p[:, i : i + 1],
                    )
                else:
                    nc.vector.tensor_reduce(
                        out=l1p[:, i : i + 1],
                        in_=y,
                        axis=mybir.AxisListType.X,
                        op=Alu.add,
                        apply_absolute_value=True,
                    )

            nc.sync.dma_start(out=out, in_=acc)

    nc.compile()
    _CACHE[key] = nc
    return nc


def _shard(pre, gt, mask):
    in_maps = []
    for i in range(N_CORES):
        sl = slice(i * BPC, (i + 1) * BPC)
        p = np.ascontiguousarray(pre[sl], dtype=np.float32).reshape(P, HW)
        g = np.ascontiguousarray(gt[sl], dtype=np.float32).reshape(P, HW)
        pgin = np.empty((P, 2 * HW), dtype=np.float32)
        for s, o in zip(SIZES, OFFS):
            pgin[:, 2 * o : 2 * o + s] = p[:, o : o + s]
            pgin[:, 2 * o + s : 2 * o + 2 * s] = -g[:, o : o + s]
        in_maps.append(
            {
                "pgin": pgin,
                "mask": np.ascontiguousarray(mask[sl], dtype=np.float32).reshape(P, HW),
            }
        )
    return in_maps


def _combine(results, batch_size):
    total = np.float32(0.0)
    for r in results:
        o = np.asarray(r["out"], dtype=np.float32)
        l1 = o[:, :NT].sum(axis=1, dtype=np.float32) + o[:, NT + NCC :].sum(
            axis=1, dtype=np.float32
        )
        ct = o[:, NT : NT + NCC].sum(axis=1, dtype=np.float32)
        total += (l1 / np.maximum(ct, np.float32(1.0))).sum(dtype=np.float32)
    return np.asarray(total / np.float32(int(batch_size)), dtype=np.float32)


def run(pre, gt, mask, batch_size, trace=False, **bass_kwargs):
    from concourse.bass_utils import run_bass_kernel_spmd

    nc = _build()
    in_maps = _shard(np.asarray(pre), np.asarray(gt), np.asarray(mask))
    res = run_bass_kernel_spmd(
        nc, in_maps, list(range(N_CORES)), trace=trace, **bass_kwargs
    )
    loss = _combine(res.results, batch_size)
    return loss, res


def kernel(pre, gt, mask, batch_size):
    loss, _ = run(pre, gt, mask, batch_size)
    return loss



# revision 4
# speedup vs baseline: 1.0071x; 1.0071x over previous
"""Masked L1 loss (per-(b,c) normalized) on 8 Trainium2 NeuronCores — v6.

Data-parallel over batch: core i takes batches [2i, 2i+2) = 128 (b,c) pairs.
The kernel is DMA-dest-byte bound; the whole pipeline runs in fp8
(3B/col of dest traffic vs 6B/col for a bf16 design):

  - Host packs pre -> e4m3, ng = -gt -> e4m3, maskC = 0x7F/0x00 bytes, in a
    TRANSPOSED layout [p = hw%128, col = chunk*128 + bc] so the PE can
    reduce over partitions.
  - sd = pre8 + ng8 happens inside the DMA engine: pre streams over SP
    HWDGE, ng accumulates via gpsimd CCE (fp8 adds, bit-exact RTN). The
    CCE datapath takes <= 2048 elements per descriptor, so ng lives in a
    padded [128, 8, 2112] HBM layout whose 2048B runs cannot merge during
    AP balancing (a flat layout re-fuses into oversize descriptors and
    wedges the device; so does the ACT HWDGE queue — everything HWDGE
    rides SP).
  - y = |pre-gt|*mask via ONE DVE tensor_tensor: bitwise AND with maskC on
    uint16-bitcast views (clears sign where mask=0x7F, zeroes elsewhere)
    at 2x DVE throughput.
  - m01 = maskC AND 0x3838 -> exact 1.0/0.0 fp8 for the count (4x-mode
    tensor_scalar).
  - PE does ALL reductions: per 128-col chunk, a ones-matmul accumulates
    out[bc] += sum_p w[p, bc] into PSUM; separate l1 / count groups.
  - Schedule: mc0 first (a big transfer hides the 2nd DMA's seq latency),
    then a small pre0 so the serial SWDGE gen chain (gated on pre_i
    completion sems, ~1.04us/gen) starts early; tiles taper so the
    post-stream tail is one small AND + matmul + PSUM copy + out DMA.
  - Host: loss = sum(l1 / max(cnt, 1)) / B.  fp8 end-to-end rel err ~2e-3
    (tolerance 2e-2).
"""

import os
import sys

os.environ.setdefault("NEURON_RT_RESET_CORES", "1")

if "/opt/trn_rl_repo" not in sys.path:
    sys.path.insert(0, "/opt/trn_rl_repo")

import numpy as np

B, C, H, W = 16, 64, 128, 128
N_CORES = 8
BPC = B // N_CORES          # batches per core = 2
NBC = BPC * C               # (b,c) pairs per core = 128 -> PE output partition
HW = H * W                  # 16384
P = 128                     # SBUF partitions = hw % 128
NCH = HW // P               # 128 chunks of the hw axis

SIZES = [2048] * 8   # uniform CCE-block tiles, sum = HW
# uniform tiles: each ng transfer (728ns) outpaces its AND (594ns), so the
# AND pipeline is arrival-limited and drains ~1.5us after the last transfer.
PROC = [0, 1, 2, 3, 4, 5, 6, 7]
OFFS = [sum(SIZES[:i]) for i in range(len(SIZES))]
NT = len(SIZES)
assert sum(SIZES) == HW
BLK = 2048                  # max elements per CCE descriptor
NBLK = HW // BLK            # 8 CCE blocks
STRIDE = 2112               # padded ng block stride: blocks can't merge into
                            # >2048B descriptors during AP balancing

_CACHE = {}


def _build():
    key = "nc"
    if key in _CACHE:
        return _CACHE[key]

    import concourse.bacc as bacc
    import concourse.bass as bass
    import concourse.mybir as mybir
    from concourse.tile import TileContext

    f32 = mybir.dt.float32
    u16 = mybir.dt.uint16
    fp8 = mybir.dt.float8e4
    Alu = mybir.AluOpType

    nc = bacc.Bacc(
        "TRN2",
        target_bir_lowering=False,
        debug=False,
        enable_asserts=False,
        num_devices=N_CORES,
    )

    pre_d = nc.dram_tensor("pre8", [P, HW], fp8, kind="ExternalInput").ap()
    ng_d = nc.dram_tensor("ng8", [P, NBLK, STRIDE], fp8, kind="ExternalInput").ap()
    mc_d = nc.dram_tensor("maskC", [P, HW], fp8, kind="ExternalInput").ap()
    out_d = nc.dram_tensor("out", [P, 2], f32, kind="ExternalOutput").ap()

    with TileContext(nc) as tc:
        with (
            tc.tile_pool(name="res", bufs=1) as res,
            tc.tile_pool(name="ps", bufs=1, space=bass.MemorySpace.PSUM) as ps,
        ):
            ones = res.tile([P, 1], fp8, tag="ones")
            nc.gpsimd.memset(ones[:], 1.0)
            mc = res.tile([P, HW], fp8, tag="mc")
            m01 = res.tile([P, HW], fp8, tag="m01")
            outt = res.tile([P, 2], f32, tag="outt")
            accL = ps.tile([P, 1], f32, tag="accL")
            accC = ps.tile([P, 1], f32, tag="accC")
            sds = [
                res.tile([P, s], fp8, tag=f"sd{i}", name=f"sd{i}")
                for i, s in enumerate(SIZES)
            ]
            ys = [
                res.tile([P, s], fp8, tag=f"y{i}", name=f"y{i}")
                for i, s in enumerate(SIZES)
            ]

            HALF = HW // 2
            # SP queue order: mc0, pre0..pre7, mc1, out.  mc0's long transfer
            # hides the next DMA's seq+DGE latency; pre0 is small so the Pool
            # gen chain (gated on pre0's completion sem) starts early.
            nc.sync.dma_start(out=mc[:, 0:HALF], in_=mc_d[:, 0:HALF])
            nc.sync.dma_start(out=mc[:, HALF:HW], in_=mc_d[:, HALF:HW])
            for i in PROC:
                s, o = SIZES[i], OFFS[i]
                nc.sync.dma_start(out=sds[i][:], in_=pre_d[:, o : o + s])

            # mask-derived work: m01 on DVE (4096-col chunks so the legacy
            # scheduler + 4-deep engine bypass window can slot them by actual
            # readiness), count chain on PE.
            for a in range(0, HW, 4096):
                nc.vector.tensor_scalar(
                    out=m01[:, a : a + 4096].bitcast(u16),
                    in0=mc[:, a : a + 4096].bitcast(u16),
                    scalar1=0x3838,
                    scalar2=None,
                    op0=Alu.bitwise_and,
                )
            for c in range(NCH):
                nc.tensor.matmul(
                    accC[:],
                    m01[:, c * P : (c + 1) * P],
                    ones[:],
                    start=(c == 0),
                    stop=(c == NCH - 1),
                )
            nc.vector.tensor_copy(out=outt[:, 1:2], in_=accC[:])

            ci = 0  # global chunk cursor for the l1 group
            for i in PROC:
                s, o = SIZES[i], OFFS[i]
                # one CCE accumulate per tile; strided src keeps runs <= 2048B
                b0, r0 = divmod(o, BLK)
                if r0 == 0 and s % BLK == 0:
                    ng_src = ng_d[:, b0 : b0 + s // BLK, 0:BLK]
                else:
                    assert r0 + s <= BLK, "partial tile must stay in one block"
                    ng_src = ng_d[:, b0 : b0 + 1, r0 : r0 + s]
                nc.gpsimd.dma_start(out=sds[i][:], in_=ng_src, accum_op=Alu.add)
                for a in range(0, s, 2048):
                    e = min(a + 2048, s)
                    nc.vector.tensor_tensor(
                        out=ys[i][:, a:e].bitcast(u16),
                        in0=sds[i][:, a:e].bitcast(u16),
                        in1=mc[:, o + a : o + e].bitcast(u16),
                        op=Alu.bitwise_and,
                    )
                for c in range(s // P):
                    nc.tensor.matmul(
                        accL[:],
                        ys[i][:, c * P : (c + 1) * P],
                        ones[:],
                        start=(ci == 0),
                        stop=(ci == NCH - 1),
                    )
                    ci += 1

            nc.vector.tensor_copy(out=outt[:, 0:1], in_=accL[:])
            nc.sync.dma_start(out=out_d, in_=outt[:])

    nc.compile()
    _CACHE[key] = nc
    return nc


def _transpose_pack(a):
    """[128 bc, 16384 hw] -> [128 p, 16384 (chunk*128+bc)] with hw=c*128+p."""
    return np.ascontiguousarray(
        a.reshape(NBC, NCH, P).transpose(2, 1, 0).reshape(P, HW)
    )


def _shard(pre, gt, mask):
    import ml_dtypes

    E4 = ml_dtypes.float8_e4m3
    in_maps = []
    for i in range(N_CORES):
        sl = slice(i * BPC, (i + 1) * BPC)
        p = pre[sl].reshape(NBC, HW).astype(E4)
        g = (-gt[sl].reshape(NBC, HW)).astype(E4)
        gp = np.zeros((P, NBLK, STRIDE), dtype=E4)
        gp[:, :, 0:BLK] = _transpose_pack(g).reshape(P, NBLK, BLK)
        m = (mask[sl].reshape(NBC, HW) != 0).astype(np.uint8) * np.uint8(0x7F)
        in_maps.append(
            {
                "pre8": _transpose_pack(p),
                "ng8": gp,
                "maskC": _transpose_pack(m).view(E4),
            }
        )
    return in_maps


def _combine(results, batch_size):
    total = np.float32(0.0)
    for r in results:
        o = np.asarray(r["out"], dtype=np.float32)
        l1, ct = o[:, 0], o[:, 1]
        total += (l1 / np.maximum(ct, np.float32(1.0))).sum(dtype=np.float32)
    return np.asarray(total / np.float32(int(batch_size)), dtype=np.float32)


def _sane(results):
    """Reject transient device glitches: l1 must be finite and >= 0, counts
    integral in [0, HW].  (A healthy run is bit-deterministic.)"""
    for r in results:
        o = np.asarray(r["out"], dtype=np.float32)
        if not np.all(np.isfinite(o)):
            return False
        l1, ct = o[:, 0], o[:, 1]
        if (l1 < 0).any() or (ct < 0).any() or (ct > HW).any():
            return False
        if not np.all(ct == np.round(ct)):
            return False
    return True


def run(pre, gt, mask, batch_size, trace=False, **bass_kwargs):
    from concourse.bass_utils import run_bass_kernel_spmd

    nc = _build()
    in_maps = _shard(np.asarray(pre), np.asarray(gt), np.asarray(mask))
    res = None
    for attempt in range(3):
        res = run_bass_kernel_spmd(
            nc, in_maps, list(range(N_CORES)), trace=trace, **bass_kwargs
        )
        if _sane(res.results):
            break
    loss = _combine(res.results, batch_size)
    return loss, res


def kernel(pre, gt, mask, batch_size):
    loss, _ = run(pre, gt, mask, batch_size)
    return loss
